# revision 1
# baseline (speedup 1.0000x reference)
"""Trainium2 Bass kernel for a 2-layer GAT (PyG GATConv semantics).

Strategy (8 NeuronCores, SPMD, 2 launches = 1 per GAT layer):
  - Destinations sharded across cores (6272 per core, incl. padding dsts).
  - Per layer, each core builds a full node-feature table in its DRAM:
      row(node n) = [h(n) | alpha_src(n) | alpha_dst(n)] in fp16,
    where h = x @ W, alpha_* = x @ (W @ att_*^T)  (PE matmuls). The table is
    split at row 32768 (tableA/tableB) to fit dma_gather's int16 indices.
  - Edges are routed by destination on the host. The Pool engine's per-row
    descriptor-generation cost (~5ns/row) dominates, so the layout minimizes
    gathered rows:
      * self-loops and destination rows are never gathered -- each core
        recomputes its own nodes' rows with 2*n_tiles extra matmuls over
        host-permuted xT copies (OWN_A / OWN_B);
      * the A-half and B-half slot grids use INDEPENDENT destination
        orderings (each degree-sorted, so both grids pad tightly); the
        B-side partial sums/denominators are written to DRAM and re-aligned
        to A-order with one extra 6272-row gather per layer.
  - Softmax is deferred: unnormalized w = exp(lrelu(e)) weights (ACT engine
    computes lrelu+exp) scale the gathered h (DVE); identity matmuls (PE)
    accumulate slot blocks per destination; the final combine divides by the
    summed denominator, adds bias, applies relu.
  - Gathers round-robin over 4 SWDGE queues; grids are padded to a uniform
    slot count per group of tiles so the DVE chain runs one fused op per
    group.
"""

import sys

for _p in ("/opt/trn_rl_repo", "/root/.axon_site/_ro/trn_rl_repo"):
    if _p not in sys.path:
        sys.path.insert(0, _p)

import inspect
import textwrap
from contextlib import ExitStack

import numpy as np

import os

import concourse.bass as _bassmod
import concourse.tile as tile
from concourse import bacc, mybir
from concourse.bass_utils import run_bass_kernel_spmd

# set GAT_TRACE=1 to profile each launch; exec times land in LAST_EXEC_NS
LAST_EXEC_NS = []
LAST_RES = []  # full BassKernelResults when tracing (trace paths, profile json)

CFG = {
    "group": 4,  # max tiles per gather call / fused DVE op
    "blk_budget": 96,  # max slot blocks per group (SBUF bound)
    "gpool_bufs": 2,
    "epool_bufs": 3,
    "bpsum_bufs": 2,
    "opsum_bufs": 3,
    "build_bufs": 3,
    "single_packet": False,
    "n_queues": 4,  # SWDGE queues (1-4); gathers round-robin across them
    "dma_scratch": 16384,
    "xchunk": 16,
    "wchunk": 8,
    "act_copies": True,
}

f32 = mybir.dt.float32
f16 = mybir.dt.float16
i16 = mybir.dt.int16

P = 128
NEG_SLOPE = 0.2
NEG_BIG = -60000.0  # alpha_src of pad rows: exp(lrelu(-60000)) == 0
HALF = 32768  # dma_gather int16 index window (rows per table view)


def _patch_swdge_lanes(n_queues):
    """Give each SWDGE queue dedicated DMASW sem lanes.

    Tile assigns the 8 DMASW completion-sem lanes round-robin over Pool DMA
    instructions regardless of queue_num. Queues drain independently, so a
    lane shared by two queues can complete out of order (the sim rejects
    this). Pin lane = queue_num * (8/n_queues) + toggle so each lane's FIFO
    matches one queue's FIFO.
    """
    import concourse.tile_sem_assignment as tsa

    if getattr(tsa.TileClockTick._assign_tick, "_gat_queues", None) == n_queues:
        return
    orig = getattr(tsa.TileClockTick._assign_tick, "_gat_orig", None) or \
        tsa.TileClockTick._assign_tick
    lanes_per_q = tsa.NUM_SWDGE_GLOBAL_SEMS // n_queues

    def patched(self, inst):
        q = getattr(inst, "queue_num", None)
        if (
            q is not None
            and inst.engine == mybir.EngineType.Pool
            and isinstance(inst, tsa.DMAInst)
        ):
            tog = getattr(self, "_gat_qtog", None)
            if tog is None:
                tog = self._gat_qtog = {}
            t = tog.get(q, 0)
            tog[q] = (t + 1) % lanes_per_q
            self.next_sw_dma_idx = q * lanes_per_q + t
        return orig(self, inst)

    patched._gat_queues = n_queues
    patched._gat_orig = orig
    tsa.TileClockTick._assign_tick = patched


def _patch_dma_gather():
    """Relax the %256 elem-size assert (the ucode only needs it for transpose)."""
    if getattr(_bassmod.BassGpSimd.dma_gather, "_gat_patched", False):
        return
    src = inspect.getsource(_bassmod.BassGpSimd.dma_gather)
    old = """        assert (
            elem_size_bytes > 0 and elem_size_bytes % 256 == 0
        )  # transpose restriction"""
    new = """        assert elem_size_bytes > 0
        if transpose:
            assert elem_size_bytes % 256 == 0"""
    assert old in src, "dma_gather source changed; patch needs updating"
    src = textwrap.dedent(src.replace(old, new))
    ns = dict(_bassmod.__dict__)
    exec(compile(src, "<dma_gather_patched>", "exec"), ns)
    ns["dma_gather"]._gat_patched = True
    _bassmod.BassGpSimd.dma_gather = ns["dma_gather"]


# ---------------------------------------------------------------- host routing


class EdgePlan:
    """Destination-sharded edge routing; A/B halves independently packed."""

    def __init__(self, src, dst, n_nodes, n_cores, half_n, group,
                 blk_budget=96):
        self.n_nodes = n_nodes
        self.n_cores = n_cores
        self.half_n = half_n  # src < half_n gathers via table view A
        self.half_base = half_n + 1  # B table view starts at this row
        assert self.half_base <= HALF and n_nodes - half_n < HALF
        self.dpc = int(np.ceil(n_nodes / n_cores / P)) * P  # dsts per core
        self.n_tiles = self.dpc // P
        self.group = group

        src = np.asarray(src, dtype=np.int64)
        dst = np.asarray(dst, dtype=np.int64)

        self.cores = []  # dicts with per-core routing state
        for c in range(n_cores):
            lo, hi = c * self.dpc, (c + 1) * self.dpc
            # data edges only (incl. any explicit (d,d) edges); the loop edge
            # the reference APPENDS per node is handled via OWN_A on-device.
            m = (dst >= lo) & (dst < hi)
            d_loc = (dst[m] - lo).astype(np.int32)
            s = src[m].astype(np.int32)
            is_b = (s >= half_n).astype(np.int32)
            order = np.lexsort((s, is_b, d_loc))
            d_loc, s, is_b = d_loc[order], s[order], is_b[order]
            degA = np.bincount(d_loc[is_b == 0], minlength=self.dpc)
            degB = np.bincount(d_loc[is_b == 1], minlength=self.dpc)
            deg = degA + degB
            offs = np.zeros(self.dpc + 1, np.int64)
            np.cumsum(deg, out=offs[1:])
            permA = np.argsort(-degA, kind="stable").astype(np.int32)
            permB = np.argsort(-degB, kind="stable").astype(np.int32)
            invB = np.empty(self.dpc, np.int32)
            invB[permB] = np.arange(self.dpc, dtype=np.int32)
            self.cores.append(dict(
                degA=degA, degB=degB, srcs=s, offs=offs,
                permA=permA, permB=permB,
                rb=invB[permA],  # B-row of the dst at A-position j
            ))

        # SPMD-uniform per-tile slot counts (max over cores)
        nt = self.n_tiles
        self.LA = np.zeros(nt, np.int64)
        self.LB = np.zeros(nt, np.int64)
        for st in self.cores:
            pa = st["degA"][st["permA"]].reshape(nt, P)
            pb = st["degB"][st["permB"]].reshape(nt, P)
            np.maximum(self.LA, pa.max(axis=1), out=self.LA)
            np.maximum(self.LB, pb.max(axis=1), out=self.LB)
        # adaptive grouping: uniform slot count per group, bounded group
        # block count (SBUF budget) -- splits the fat degree-sorted head
        self.groups = []  # (g0, T)
        budget = blk_budget
        t0 = 0
        while t0 < nt:
            T = 1
            while (
                T < group
                and t0 + T < nt
                and (T + 1) * int(self.LA[t0:t0 + T + 1].max()) <= budget
                and (T + 1) * int(self.LB[t0:t0 + T + 1].max()) <= budget
            ):
                T += 1
            self.groups.append((t0, T))
            t0 += T
        self.LAg = {g0: int(self.LA[g0:g0 + T].max()) for g0, T in self.groups}
        self.LBg = {g0: int(self.LB[g0:g0 + T].max()) for g0, T in self.groups}

        # idx column layout: B grids, then A grids, then the realign segment
        col = 0
        self.colB, self.colA = {}, {}
        for g0, T in self.groups:
            self.colB[g0] = col
            col += T * self.LBg[g0] * 8
        for g0, T in self.groups:
            self.colA[g0] = col
            col += T * self.LAg[g0] * 8
        self.colR = col
        col += nt * 8
        self.W = col

    def n_table_rows(self, n_node_tiles):
        padb = 1 + n_node_tiles * P
        return padb + 1, padb

    @staticmethod
    def _wrap16(flat):
        w = np.zeros((16, flat.size // 16), np.int16)
        ar = np.arange(flat.size)
        w[ar % 16, ar // 16] = flat
        return np.tile(w, (8, 1))

    def build_idx(self, core, padb_row):
        """int16 idx array [128, W] matching the device's gather emission."""
        st = self.cores[core]
        degA, degB, srcs, offs = st["degA"], st["degB"], st["srcs"], st["offs"]
        permA, permB = st["permA"], st["permB"]
        hb = self.half_base
        nt = self.n_tiles
        grp = self.group
        segs = []
        for g0, tn in self.groups:  # B grids
            L = self.LBg[g0]
            grid = np.full((tn, L, P), padb_row - hb, np.int32)
            for t in range(tn):
                dsts = permB[(g0 + t) * P:(g0 + t + 1) * P]
                for j, d in enumerate(dsts):
                    db = degB[d]
                    if db:
                        o = offs[d] + degA[d]
                        grid[t, :db, j] = srcs[o:o + db] + 1 - hb
            segs.append(self._wrap16(grid.reshape(-1)))
        for g0, tn in self.groups:  # A grids
            L = self.LAg[g0]
            grid = np.zeros((tn, L, P), np.int32)  # pad -> row 0 (PAD_A)
            for t in range(tn):
                dsts = permA[(g0 + t) * P:(g0 + t + 1) * P]
                for j, d in enumerate(dsts):
                    da = degA[d]
                    if da:
                        o = offs[d]
                        grid[t, :da, j] = srcs[o:o + da] + 1
            segs.append(self._wrap16(grid.reshape(-1)))
        segs.append(self._wrap16(st["rb"].astype(np.int32)))
        return np.concatenate(segs, axis=1)

    def xtp(self, core, x_t, order):
        """[128, dpc] f16: x^T columns for this core's nodes in A/B order."""
        st = self.cores[core]
        perm = st["permA"] if order == "A" else st["permB"]
        node = core * self.dpc + perm
        valid = node < self.n_nodes
        out = np.zeros((x_t.shape[0], self.dpc), np.float16)
        out[:, valid] = x_t[:, node[valid]]
        return out

    def unpermute(self, core_outs, fout):
        full = np.zeros((self.n_nodes, fout), np.float32)
        for c, arr in enumerate(core_outs):
            node = c * self.dpc + self.cores[c]["permA"]
            m = node < self.n_nodes
            full[node[m]] = arr[m]
        return full


# ------------------------------------------------------------- device program


def build_layer_program(plan: EdgePlan, fin, n_heads, ch, relu, n_cores):
    """One GAT layer: table build + edge aggregation. Returns compiled Bacc."""
    _patch_dma_gather()
    _patch_swdge_lanes(CFG["n_queues"])
    outf = n_heads * ch
    rowv = outf + 2 * n_heads  # [h | alpha_src | alpha_dst]
    pitch = 1 << int(np.ceil(np.log2(rowv)))  # fp16 row pitch (values)
    assert pitch * 2 % 256 == 0
    browv = outf + n_heads  # oB row: [num | den]
    bpitch = 1 << int(np.ceil(np.log2(browv)))
    assert bpitch * 2 % 256 == 0
    n_node_tiles = int(np.ceil(plan.n_nodes / P))
    nodes_pad = n_node_tiles * P
    n_rows, padb_row = plan.n_table_rows(n_node_tiles)
    nt = plan.n_tiles
    H, C = n_heads, ch

    nc = bacc.Bacc(
        "TRN2",
        target_bir_lowering=False,
        debug=False,
        num_devices=n_cores,
        dynamic_dma_scratch_size=CFG["dma_scratch"],
        num_swdge_queues=CFG["n_queues"],
    )
    xT = nc.dram_tensor("xT", [P, nodes_pad], f16, kind="ExternalInput").ap()
    xTpA = nc.dram_tensor("xTpA", [P, plan.dpc], f16, kind="ExternalInput").ap()
    xTpB = nc.dram_tensor("xTpB", [P, plan.dpc], f16, kind="ExternalInput").ap()
    wext = nc.dram_tensor("wext", [P, rowv], f16, kind="ExternalInput").ap()
    bias = nc.dram_tensor("bias", [P, outf], f32, kind="ExternalInput").ap()
    ident_in = nc.dram_tensor("ident", [P, P], f16, kind="ExternalInput").ap()
    idx_in = nc.dram_tensor("idx", [P, plan.W], i16, kind="ExternalInput").ap()
    out = nc.dram_tensor("out", [plan.dpc, outf], f32, kind="ExternalOutput").ap()
    hb = plan.half_base
    tableA = nc.dram_tensor("tableA", [hb, pitch], f16)
    tableB = nc.dram_tensor("tableB", [n_rows - hb, pitch], f16)
    oBtab = nc.dram_tensor("oBtab", [plan.dpc, bpitch], f16)

    XCH = CFG["xchunk"]
    qrr = [0]

    def next_q():
        q = qrr[0] % CFG["n_queues"]
        qrr[0] += 1
        return q

    with tile.TileContext(nc) as tc, ExitStack() as ctx:
        const = ctx.enter_context(tc.tile_pool(name="const", bufs=1))
        build = ctx.enter_context(tc.tile_pool(name="build", bufs=CFG["build_bufs"]))
        bpsum = ctx.enter_context(
            tc.tile_pool(name="bpsum", bufs=CFG["bpsum_bufs"], space="PSUM")
        )
        gpool = ctx.enter_context(tc.tile_pool(name="gpool", bufs=CFG["gpool_bufs"]))
        epool = ctx.enter_context(tc.tile_pool(name="epool", bufs=CFG["epool_bufs"]))
        opsum = ctx.enter_context(
            tc.tile_pool(name="opsum", bufs=CFG["opsum_bufs"], space="PSUM")
        )

        ident = const.tile([P, P], f16)
        nc.sync.dma_start(out=ident[:], in_=ident_in[:])
        wext_sb = const.tile([P, rowv], f16)
        nc.sync.dma_start(out=wext_sb[:], in_=wext[:])
        bias_sb = const.tile([P, outf], f32)
        nc.sync.dma_start(out=bias_sb[:], in_=bias[:])
        idx_sb = const.tile([P, plan.W], i16)
        nc.sync.dma_start(out=idx_sb[:], in_=idx_in[:])
        xtpa_sb = const.tile([P, plan.dpc], f16)
        nc.sync.dma_start(out=xtpa_sb[:], in_=xTpA[:])
        xtpb_sb = const.tile([P, plan.dpc], f16)
        nc.sync.dma_start(out=xtpb_sb[:], in_=xTpB[:])

        # ---- OWN_B: alpha_dst of own dsts in B order (one psum bank, 1 copy)
        OWNB = const.tile([P, nt * H], f16)
        ps_b = bpsum.tile([P, max(nt * H, rowv)], f32, space="PSUM", tag="bps")
        for k in range(nt):
            nc.tensor.matmul(
                out=ps_b[:, k * H:(k + 1) * H],
                lhsT=xtpb_sb[:, k * P:(k + 1) * P],
                rhs=wext_sb[:, outf + H:outf + 2 * H],
                start=True, stop=True,
            )
        nc.vector.tensor_copy(out=OWNB[:], in_=ps_b[:, :nt * H])

        # ---- OWN_A: full rows of own dsts in A order
        OWNA = const.tile([P, nt * rowv], f16)
        for k in range(nt):
            ps = bpsum.tile([P, max(nt * H, rowv)], f32, space="PSUM", tag="bps")
            nc.tensor.matmul(
                out=ps[:, :rowv],
                lhsT=xtpa_sb[:, k * P:(k + 1) * P],
                rhs=wext_sb[:],
                start=True, stop=True,
            )
            dst_sl = OWNA[:, k * rowv:(k + 1) * rowv]
            if CFG["act_copies"] and k % 2 == 1:
                nc.scalar.copy(dst_sl, ps[:, :rowv])
            else:
                nc.vector.tensor_copy(out=dst_sl, in_=ps[:, :rowv])
        OWNA3 = OWNA[:].rearrange("p (t v) -> p t v", t=nt, v=rowv)

        # ---- WS: self-loop weights exp(lrelu(a_src + a_dst)) in A order, f32
        WS = const.tile([P, nt * H], f32)
        WS3 = WS[:].rearrange("p (t h) -> p t h", t=nt, h=H)
        eS = epool.tile([P, nt * H], f16, tag="eS")
        eS3 = eS[:].rearrange("p (t h) -> p t h", t=nt, h=H)
        nc.vector.tensor_tensor(
            out=eS3, in0=OWNA3[:, :, outf:outf + H],
            in1=OWNA3[:, :, outf + H:outf + 2 * H], op=mybir.AluOpType.add,
        )
        nc.vector.scalar_tensor_tensor(
            out=eS[:], in0=eS[:], scalar=NEG_SLOPE, in1=eS[:],
            op0=mybir.AluOpType.mult, op1=mybir.AluOpType.max,
        )
        nc.scalar.activation(WS[:], eS[:], mybir.ActivationFunctionType.Exp)

        # ---- table build, rows >= hb (tableB) first so B gathers start early
        WCH = CFG["wchunk"]
        assert XCH % WCH == 0
        chunk_order = list(range(0, n_node_tiles, XCH))
        bsplit = (hb - 1) // P // XCH  # first chunk touching tableB
        chunk_order = chunk_order[bsplit:] + chunk_order[:bsplit]
        for c0 in chunk_order:
            cn = min(XCH, n_node_tiles - c0)
            xchunk = build.tile([P, XCH * P], f16, tag="xchunk")
            nc.sync.dma_start(
                out=xchunk[:, : cn * P], in_=xT[:, c0 * P:(c0 + cn) * P]
            )
            for w0 in range(0, cn, WCH):
                wn = min(WCH, cn - w0)
                row_sb = build.tile([P, WCH * rowv], f16, tag="rowsb")
                for t in range(w0, w0 + wn):
                    ps = bpsum.tile([P, max(nt * H, rowv)], f32, space="PSUM",
                                    tag="bps")
                    nc.tensor.matmul(
                        out=ps[:, :rowv],
                        lhsT=xchunk[:, t * P:(t + 1) * P],
                        rhs=wext_sb[:],
                        start=True, stop=True,
                    )
                    dst_sl = row_sb[:, (t - w0) * rowv:(t - w0 + 1) * rowv]
                    if CFG["act_copies"] and t % 2 == 1:
                        nc.scalar.copy(dst_sl, ps[:, :rowv])
                    else:
                        nc.vector.tensor_copy(out=dst_sl, in_=ps[:, :rowv])
                r0 = 1 + (c0 + w0) * P
                r1 = r0 + wn * P
                for lo, hi, tab, base in (
                    (r0, min(r1, hb), tableA, 0),
                    (max(r0, hb), r1, tableB, hb),
                ):
                    if lo >= hi:
                        continue
                    t0 = (lo - r0) // P
                    tn = (hi - lo + P - 1) // P
                    if (lo - r0) % P == 0 and (hi - lo) % P == 0:
                        nc.sync.dma_start(
                            out=tab[lo - base:hi - base, :rowv].rearrange(
                                "(t p) v -> p t v", t=tn
                            ),
                            in_=row_sb[:, t0 * rowv:(t0 + tn) * rowv].rearrange(
                                "p (t v) -> p t v", t=tn
                            ),
                        )
                    else:
                        rr = lo
                        while rr < hi:
                            tt = (rr - r0) // P
                            po = (rr - r0) % P
                            rn = min(P - po, hi - rr)
                            nc.sync.dma_start(
                                out=tab[rr - base:rr - base + rn, :rowv],
                                in_=row_sb[po:po + rn, tt * rowv:(tt + 1) * rowv],
                            )
                            rr += rn

        # ---- pad rows (row 0 = PAD_A, row padb_row = PAD_B)
        padrow = build.tile([1, pitch], f16, tag="padrow")
        nc.vector.memset(padrow[:], 0.0)
        nc.vector.memset(padrow[:, outf:outf + H], NEG_BIG)
        nc.sync.dma_start(
            out=tableB[padb_row - hb:padb_row - hb + 1, :rowv],
            in_=padrow[:, :rowv],
        )
        nc.sync.dma_start(out=tableA[0:1, :rowv], in_=padrow[:, :rowv])

        grp = plan.group

        def gather(g_out, g_tab, g_rowv, g_pitch, col, n_rows_g):
            nc.gpsimd.dma_gather(
                out_ap=g_out,
                in_ap=g_tab[:, :g_rowv],
                idxs_ap=idx_sb[:, col:col + n_rows_g * 8],
                num_idxs=n_rows_g * P,
                num_idxs_reg=n_rows_g * P,
                elem_size=g_rowv,
                elem_step=g_pitch,
                single_packet=CFG["single_packet"],
                queue_num=next_q(),
            )

        def att_weights(G4, G3, adview, T, L, tag):
            """W = exp(lrelu(gathered_alpha_src + alpha_dst)), f16 [P,T,L,H]."""
            E = epool.tile([P, T * L * H], f16, tag=f"E{tag}")
            E4 = E[:].rearrange("p (t l h) -> p t l h", t=T, l=L, h=H)
            nc.vector.tensor_tensor(
                out=E4,
                in0=G4[:, :, :, outf:outf + H],
                in1=adview.unsqueeze(2).to_broadcast([P, T, L, H]),
                op=mybir.AluOpType.add,
            )
            W = epool.tile([P, T * L * H], f16, tag=f"W{tag}")
            nc.vector.scalar_tensor_tensor(
                out=W[:], in0=E[:], scalar=NEG_SLOPE, in1=E[:],
                op0=mybir.AluOpType.mult, op1=mybir.AluOpType.max,
            )
            nc.scalar.activation(W[:], W[:], mybir.ActivationFunctionType.Exp)
            W4 = W[:].rearrange("p (t l h) -> p t l h", t=T, l=L, h=H)
            den = epool.tile([P, T * H], f32, tag=f"den{tag}")
            den3 = den[:].rearrange("p (t h) -> p t h", t=T, h=H)
            nc.vector.tensor_reduce(
                out=den3, in_=W4.transpose([0, 1, 3, 2]),
                axis=mybir.AxisListType.X, op=mybir.AluOpType.add,
            )
            # scale gathered h by w IN PLACE (h columns of G)
            gh4 = G3[:, :, :outf].rearrange("p b (c h) -> p b c h", c=C, h=H)
            nc.vector.tensor_tensor(
                out=gh4,
                in0=gh4,
                in1=W[:].rearrange("p (b h) -> p b h", b=T * L, h=H)
                .unsqueeze(2).to_broadcast([P, T * L, C, H]),
                op=mybir.AluOpType.mult,
            )
            return G3[:, :, :outf], den3

        # ---- B phase: partial sums in B order -> oBtab
        for g0, T in plan.groups:
            L = plan.LBg[g0]
            osb = epool.tile([P, T * browv], f16, tag="osbB")
            osb3 = osb[:].rearrange("p (t v) -> p t v", t=T, v=browv)
            if L == 0:
                nc.vector.memset(osb[:], 0.0)
            else:
                G = gpool.tile([P, T * L * rowv], f16, tag="GB")
                G4 = G[:].rearrange("p (t l v) -> p t l v", t=T, l=L, v=rowv)
                G3 = G[:].rearrange("p (b v) -> p b v", b=T * L, v=rowv)
                gather(G3, tableB, rowv, pitch, plan.colB[g0], T * L)
                adB = OWNB[:].rearrange("p (t h) -> p t h", t=nt, h=H)[
                    :, g0:g0 + T, :
                ]
                wG3, den3 = att_weights(G4, G3, adB, T, L, "B")
                for t in range(T):
                    ps = opsum.tile([P, outf], f32, space="PSUM", tag="opsB")
                    for j in range(L):
                        nc.tensor.matmul(
                            out=ps[:], lhsT=ident[:], rhs=wG3[:, t * L + j, :],
                            start=(j == 0), stop=(j == L - 1),
                        )
                    if CFG["act_copies"] and t % 2 == 1:
                        nc.scalar.copy(osb3[:, t, :outf], ps[:])
                    else:
                        nc.vector.tensor_copy(out=osb3[:, t, :outf], in_=ps[:])
                nc.vector.tensor_copy(out=osb3[:, :, outf:], in_=den3)
            nc.sync.dma_start(
                out=oBtab[g0 * P:(g0 + T) * P, :browv].rearrange(
                    "(t p) v -> p t v", t=T
                ),
                in_=osb3,
            )

        # ---- realign gather: B partials for A-ordered dsts
        RB = const.tile([P, nt * browv], f16)
        RB3 = RB[:].rearrange("p (t v) -> p t v", t=nt, v=browv)
        gather(RB3, oBtab, browv, bpitch, plan.colR, nt)

        # ---- A phase: gather, accumulate with self row, combine with B, out
        for g0, T in plan.groups:
            L = plan.LAg[g0]
            if L > 0:
                G = gpool.tile([P, T * L * rowv], f16, tag="GA")
                G4 = G[:].rearrange("p (t l v) -> p t l v", t=T, l=L, v=rowv)
                G3 = G[:].rearrange("p (b v) -> p b v", b=T * L, v=rowv)
                gather(G3, tableA, rowv, pitch, plan.colA[g0], T * L)
                adA = OWNA3[:, g0:g0 + T, outf + H:outf + 2 * H]
                wG3, den3 = att_weights(G4, G3, adA, T, L, "A")
            else:
                den = epool.tile([P, T * H], f32, tag="denA")
                den3 = den[:].rearrange("p (t h) -> p t h", t=T, h=H)
                nc.vector.memset(den[:], 0.0)

            # wOWN = h_own * w_self for this group's tiles
            wOWN = epool.tile([P, T * outf], f16, tag="wOWN")
            nc.vector.tensor_tensor(
                out=wOWN[:].rearrange("p (t c h) -> p t c h", t=T, c=C, h=H),
                in0=OWNA3[:, g0:g0 + T, :outf].rearrange(
                    "p t (c h) -> p t c h", c=C, h=H
                ),
                in1=WS3[:, g0:g0 + T, :].unsqueeze(2).to_broadcast([P, T, C, H]),
                op=mybir.AluOpType.mult,
            )
            # den_total = denA + w_self + denB(realigned)
            nc.vector.tensor_tensor(
                out=den3, in0=den3, in1=WS3[:, g0:g0 + T, :],
                op=mybir.AluOpType.add,
            )
            nc.vector.tensor_tensor(
                out=den3, in0=den3, in1=RB3[:, g0:g0 + T, outf:],
                op=mybir.AluOpType.add,
            )
            rec = epool.tile([P, T * H], f32, tag="rec")
            nc.vector.reciprocal(
                rec[:], den3.rearrange("p t h -> p (t h)")
            )
            rec3 = rec[:].rearrange("p (t h) -> p t h", t=T, h=H)

            osb = epool.tile([P, T * outf], f32, tag="osbA")
            osb3 = osb[:].rearrange("p (t f) -> p t f", t=T, f=outf)
            wO3 = wOWN[:].rearrange("p (t f) -> p t f", t=T, f=outf)
            for t in range(T):
                ps = opsum.tile([P, outf], f32, space="PSUM", tag="opsA")
                nc.tensor.matmul(out=ps[:], lhsT=ident[:], rhs=wO3[:, t, :],
                                 start=True, stop=(L == 0))
                for j in range(L):
                    nc.tensor.matmul(
                        out=ps[:], lhsT=ident[:], rhs=wG3[:, t * L + j, :],
                        start=False, stop=(j == L - 1),
                    )
                # num_total = psA + numB(realigned); then * rec
                nc.vector.tensor_tensor(
                    out=osb3[:, t, :], in0=ps[:], in1=RB3[:, g0 + t, :outf],
                    op=mybir.AluOpType.add,
                )
                nc.vector.tensor_tensor(
                    out=osb3[:, t, :].rearrange("p (c h) -> p c h", c=C, h=H),
                    in0=osb3[:, t, :].rearrange("p (c h) -> p c h", c=C, h=H),
                    in1=rec3[:, t, :].unsqueeze(1).to_broadcast([P, C, H]),
                    op=mybir.AluOpType.mult,
                )
            nc.vector.tensor_tensor(
                out=osb3, in0=osb3,
                in1=bias_sb[:].unsqueeze(1).to_broadcast([P, T, outf]),
                op=mybir.AluOpType.add,
            )
            if relu:
                nc.scalar.activation(osb[:], osb[:],
                                     mybir.ActivationFunctionType.Relu)
            nc.sync.dma_start(
                out=out[g0 * P:(g0 + T) * P, :].rearrange("(t p) f -> p t f", t=T),
                in_=osb3,
            )

    nc.compile()
    return nc, padb_row, nodes_pad


# ------------------------------------------------------------------ execution


def _prep_wext(W, att_src, att_dst):
    """[fin, outf + 2H] fp16: [W (c-major cols) | W @ att_src^T | W @ att_dst^T]."""
    H, C = att_src.shape
    fin = W.shape[0]
    Wr = W.reshape(fin, H, C)
    a_s = np.einsum("fhc,hc->fh", Wr, att_src)
    a_d = np.einsum("fhc,hc->fh", Wr, att_dst)
    Wi = Wr.transpose(0, 2, 1).reshape(fin, H * C)  # (c, h) column order
    return np.concatenate([Wi, a_s, a_d], axis=1).astype(np.float16)


def _interleave_cols(v, H, C):
    return np.asarray(v, np.float32).reshape(H, C).T.reshape(H * C)


def _deinterleave(arr, H, C):
    """[n, (c h)] -> [n, (h c)]"""
    n = arr.shape[0]
    return arr.reshape(n, C, H).transpose(0, 2, 1).reshape(n, H * C)


def _xT_f16(x, nodes_pad):
    n = x.shape[0]
    xt = np.zeros((x.shape[1], nodes_pad), np.float16)
    xt[:, :n] = np.asarray(x, np.float32).T.astype(np.float16)
    return xt


def run_layer(plan, nc_bundle, x, W, att_src, att_dst, b, relu, n_cores, idx_arrs):
    nc, padb_row, nodes_pad = nc_bundle
    H, C = att_src.shape
    outf = H * C
    wext = _prep_wext(np.asarray(W, np.float32), np.asarray(att_src, np.float32),
                      np.asarray(att_dst, np.float32))
    xt = _xT_f16(x, nodes_pad)
    bias = np.broadcast_to(_interleave_cols(b, H, C), (P, outf)).copy()
    ident = np.eye(P, dtype=np.float16)
    in_maps = [
        {"xT": xt, "wext": wext, "bias": bias, "ident": ident,
         "idx": idx_arrs[c], "xTpA": plan.xtp(c, xt, "A"),
         "xTpB": plan.xtp(c, xt, "B")}
        for c in range(n_cores)
    ]
    trace = os.environ.get("GAT_TRACE", "") == "1"
    res = run_bass_kernel_spmd(nc, in_maps, list(range(n_cores)), trace=trace)
    if trace:
        LAST_EXEC_NS.append(res.exec_time_ns)
        LAST_RES.append(res)
    outs = [res.results[c]["out"] for c in range(n_cores)]
    return _deinterleave(plan.unpermute(outs, outf), H, C)


def gat_forward(x, edge_index, params, n_cores=8, half_n=HALF - 1):
    """params: (W1, as1, ad1, b1, W2, as2, ad2, b2). Returns [N, F2] fp32."""
    x = np.asarray(x, np.float32)
    n = x.shape[0]
    ei = np.asarray(edge_index)
    src = ei[0]
    dst = ei[1]

    plan = EdgePlan(src, dst, n, n_cores, half_n, CFG["group"],
                    blk_budget=CFG["blk_budget"])
    W1, as1, ad1, b1, W2, as2, ad2, b2 = params

    bundle1 = build_layer_program(plan, x.shape[1], as1.shape[0], as1.shape[1],
                                  relu=True, n_cores=n_cores)
    idx_arrs = [plan.build_idx(c, bundle1[1]) for c in range(n_cores)]
    h = run_layer(plan, bundle1, x, W1, as1, ad1, b1, True, n_cores, idx_arrs)

    bundle2 = build_layer_program(plan, h.shape[1], as2.shape[0], as2.shape[1],
                                  relu=False, n_cores=n_cores)
    assert bundle2[1] == bundle1[1]
    out = run_layer(plan, bundle2, h, W2, as2, ad2, b2, False, n_cores, idx_arrs)
    return out


def kernel(x, edge_index, W1, att_src1, att_dst1, b1, W2, att_src2, att_dst2, b2):
    params = tuple(
        np.asarray(a, np.float32)
        for a in (W1, att_src1, att_dst1, b1, W2, att_src2, att_dst2, b2)
    )
    return gat_forward(x, edge_index, params).astype(np.float32)



# revision 2
# speedup vs baseline: 3.2310x; 3.2310x over previous
"""Trainium2 Bass kernel for a 2-layer GAT (PyG GATConv semantics).

Strategy (8 NeuronCores, SPMD, 2 launches = 1 per GAT layer):
  - Destinations sharded across cores (6272 per core incl. padding dsts),
    destinations degree-sorted so per-tile slot grids pad tightly.
  - NO device-side gather. The host slot-expands the layer input:
    xTsl[:, b*128 + j] = x^T column of the source of edge slot (b, j),
    where block b = (tile t, slot level l) and partition j = destination
    lane. Slot level 0 is the self-loop (PyG add_self_loops); levels
    1..deg are the in-edges; the rest are zero-padded (masked).
  - Each 128-column block becomes one PE matmul lhsT against
    wext = [W | W@a_src^T | W@a_dst^T], producing full per-edge rows
    [h | alpha_src | alpha_dst-of-src] directly in PSUM -- the same
    trick the previous version used for self-loop rows only, now for
    every edge. PSUM blocks are copied (batched per bank) to SBUF.
  - Attention: e = alpha_src(slot) + alpha_dst(dst) (dst alphas from a
    small per-tile matmul over own columns), w = exp(lrelu(e)) * mask;
    softmax is deferred: DVE reduces w and w*h over the slot axis, then
    one reciprocal multiply normalizes; + bias (+ relu for layer 1).
  - Between layers the host assembles h1, casts to fp16 and re-expands
    the SAME slot grid (graph is static), so layer 2 is identical with
    H=1, C=64.
"""

import sys

for _p in ("/opt/trn_rl_repo", "/root/.axon_site/_ro/trn_rl_repo"):
    if _p not in sys.path:
        sys.path.insert(0, _p)

import os
from contextlib import ExitStack

import numpy as np

import concourse.tile as tile
from concourse import bacc, mybir
from concourse.bass_utils import run_bass_kernel_spmd

# set GAT_TRACE=1 to profile each launch; exec times land in LAST_EXEC_NS
LAST_EXEC_NS = []
LAST_RES = []

CFG = {
    "group": 16,       # max tiles per group
    "blk_budget": 96,  # max T*L blocks per group (SBUF bound)
    "xsl_bufs": 2,
    "gpool_bufs": 2,
    "epool_bufs": 3,
    "psum_bufs": 6,
    "opsum_bufs": 2,
}

f32 = mybir.dt.float32
f16 = mybir.dt.float16

P = 128
NEG_SLOPE = 0.2
N_NODES = 50000
N_CORES = 8


# ---------------------------------------------------------------- host routing


class SlotPlan:
    """Destination-sharded slot grid; slot 0 = self-loop, then in-edges."""

    def __init__(self, src, dst, n_nodes, n_cores, group, blk_budget):
        self.n_nodes = n_nodes
        self.n_cores = n_cores
        self.dpc = int(np.ceil(n_nodes / n_cores / P)) * P
        self.nt = self.dpc // P
        nt = self.nt

        src = np.asarray(src, dtype=np.int64)
        dst = np.asarray(dst, dtype=np.int64)

        self.cores = []
        Ls = np.zeros(nt, np.int64)
        for c in range(n_cores):
            lo, hi = c * self.dpc, (c + 1) * self.dpc
            m = (dst >= lo) & (dst < hi)
            d_loc = (dst[m] - lo).astype(np.int64)
            s = src[m].astype(np.int64)
            order = np.argsort(d_loc, kind="stable")
            d_loc, s = d_loc[order], s[order]
            deg = np.bincount(d_loc, minlength=self.dpc)
            offs = np.zeros(self.dpc + 1, np.int64)
            np.cumsum(deg, out=offs[1:])
            perm = np.argsort(-deg, kind="stable").astype(np.int64)
            self.cores.append(dict(deg=deg, offs=offs, srcs=s, perm=perm))
            pt = deg[perm].reshape(nt, P)
            np.maximum(Ls, pt.max(axis=1) + 1, out=Ls)  # +1 self slot

        # SPMD-uniform groups: (g0, T) tiles sharing slot depth Lg
        self.groups = []
        t0 = 0
        while t0 < nt:
            T = 1
            while (
                T < group and t0 + T < nt
                and (T + 1) * int(Ls[t0:t0 + T + 1].max()) <= blk_budget
            ):
                T += 1
            self.groups.append((t0, T))
            t0 += T
        self.Lg = {g0: int(Ls[g0:g0 + T].max()) for g0, T in self.groups}
        self.n_blocks = sum(T * self.Lg[g0] for g0, T in self.groups)
        self.S = self.n_blocks * P  # total slot columns per core
        # block start offset per group
        self.gblk = {}
        b = 0
        for g0, T in self.groups:
            self.gblk[g0] = b
            b += T * self.Lg[g0]

        # per-core slot->node map (-1 = pad) and validity mask
        self.col_node = []
        self.masks = []
        for c in range(n_cores):
            st = self.cores[c]
            deg, offs, srcs, perm = (st["deg"], st["offs"], st["srcs"],
                                     st["perm"])
            cn = np.full((self.n_blocks, P), -1, np.int64)
            for g0, T in self.groups:
                L = self.Lg[g0]
                b0 = self.gblk[g0]
                for t in range(T):
                    dsts = perm[(g0 + t) * P:(g0 + t + 1) * P]
                    node = c * self.dpc + dsts
                    ok = node < n_nodes
                    # slot 0: self column (pad dst -> -1 column = zeros)
                    cn[b0 + t * L, ok] = node[ok]
                    for j in range(P):
                        d = dsts[j]
                        dd = deg[d]
                        if dd:
                            o = offs[d]
                            cn[b0 + t * L + 1:b0 + t * L + 1 + dd, j] = \
                                srcs[o:o + dd]
            self.col_node.append(cn)
            mask = (cn >= 0).astype(np.float16)
            # self slots of pad dsts: keep 1 so den=exp(0)=1 (row dropped)
            for g0, T in self.groups:
                L = self.Lg[g0]
                b0 = self.gblk[g0]
                for t in range(T):
                    mask[b0 + t * L, :] = 1.0
            self.masks.append(np.ascontiguousarray(mask.T))  # [P, n_blocks]

    def expand(self, core, x_t):
        """[128, S] f16: x^T columns in slot order; pad -> 0."""
        cn = self.col_node[core].reshape(-1)
        out = np.zeros((x_t.shape[0], cn.size), np.float16)
        ok = cn >= 0
        out[:, ok] = x_t[:, cn[ok]]
        return out

    def xtown(self, core, x_t):
        """[128, dpc] f16: own dst columns (A-order) for alpha_dst."""
        st = self.cores[core]
        node = core * self.dpc + st["perm"]
        valid = node < self.n_nodes
        out = np.zeros((x_t.shape[0], self.dpc), np.float16)
        out[:, valid] = x_t[:, node[valid]]
        return out

    def unpermute(self, core_outs, fout):
        full = np.zeros((self.n_nodes, fout), np.float32)
        for c, arr in enumerate(core_outs):
            node = c * self.dpc + self.cores[c]["perm"]
            m = node < self.n_nodes
            full[node[m]] = arr[m]
        return full


# ------------------------------------------------------------- device program


def build_layer_program(plan: SlotPlan, n_heads, ch, relu, n_cores):
    """One GAT layer over host-expanded slot columns. Returns compiled Bacc."""
    outf = n_heads * ch
    rowv = outf + 2 * n_heads  # [h | alpha_src | alpha_dst]
    nt = plan.nt
    H, C = n_heads, ch
    per_bank = 512 // rowv  # f32 psum cols per bank -> blocks per copy

    nc = bacc.Bacc(
        "TRN2",
        target_bir_lowering=False,
        debug=False,
        num_devices=n_cores,
    )
    xsl = nc.dram_tensor("xsl", [P, plan.S], f16, kind="ExternalInput").ap()
    xtown = nc.dram_tensor("xtown", [P, plan.dpc], f16,
                           kind="ExternalInput").ap()
    wext = nc.dram_tensor("wext", [P, rowv], f16, kind="ExternalInput").ap()
    maskin = nc.dram_tensor("mask", [P, plan.n_blocks], f16,
                            kind="ExternalInput").ap()
    bias = nc.dram_tensor("bias", [P, outf], f32, kind="ExternalInput").ap()
    out = nc.dram_tensor("out", [plan.dpc, outf], f32,
                         kind="ExternalOutput").ap()

    with tile.TileContext(nc) as tc, ExitStack() as ctx:
        const = ctx.enter_context(tc.tile_pool(name="const", bufs=1))
        xpool = ctx.enter_context(
            tc.tile_pool(name="xpool", bufs=CFG["xsl_bufs"]))
        gpool = ctx.enter_context(
            tc.tile_pool(name="gpool", bufs=CFG["gpool_bufs"]))
        epool = ctx.enter_context(
            tc.tile_pool(name="epool", bufs=CFG["epool_bufs"]))
        bpsum = ctx.enter_context(
            tc.tile_pool(name="bpsum", bufs=CFG["psum_bufs"], space="PSUM"))
        opsum = ctx.enter_context(
            tc.tile_pool(name="opsum", bufs=CFG["opsum_bufs"], space="PSUM"))

        wext_sb = const.tile([P, rowv], f16)
        nc.sync.dma_start(out=wext_sb[:], in_=wext[:])
        bias_sb = const.tile([P, outf], f32)
        nc.sync.dma_start(out=bias_sb[:], in_=bias[:])
        mask_sb = const.tile([P, plan.n_blocks], f16)
        nc.sync.dma_start(out=mask_sb[:], in_=maskin[:])
        xtown_sb = const.tile([P, plan.dpc], f16)
        nc.sync.dma_start(out=xtown_sb[:], in_=xtown[:])

        # ---- OWND: alpha_dst of own dsts [P, nt*H] (one psum bank)
        OWND = const.tile([P, nt * H], f16)
        ps_d = opsum.tile([P, max(nt * H, outf)], f32, space="PSUM", tag="ops")
        for k in range(nt):
            nc.tensor.matmul(
                out=ps_d[:, k * H:(k + 1) * H],
                lhsT=xtown_sb[:, k * P:(k + 1) * P],
                rhs=wext_sb[:, outf + H:outf + 2 * H],
                start=True, stop=True,
            )
        nc.vector.tensor_copy(out=OWND[:], in_=ps_d[:, :nt * H])
        OWND3 = OWND[:].rearrange("p (t h) -> p t h", t=nt, h=H)

        # ---- per-group pipeline
        for g0, T in plan.groups:
            L = plan.Lg[g0]
            nblk = T * L
            b0 = plan.gblk[g0]

            xch = xpool.tile([P, nblk * P], f16, tag="xch")
            nc.sync.dma_start(
                out=xch[:], in_=xsl[:, b0 * P:(b0 + nblk) * P])

            G = gpool.tile([P, nblk * rowv], f16, tag="G")
            # build rows: one matmul per block, batched per psum bank
            b = 0
            copy_tog = 0
            while b < nblk:
                bn = min(per_bank, nblk - b)
                ps = bpsum.tile([P, 512], f32, space="PSUM", tag="bps")
                for k in range(bn):
                    nc.tensor.matmul(
                        out=ps[:, k * rowv:(k + 1) * rowv],
                        lhsT=xch[:, (b + k) * P:(b + k + 1) * P],
                        rhs=wext_sb[:],
                        start=True, stop=True,
                    )
                dst_sl = G[:, b * rowv:(b + bn) * rowv]
                if copy_tog % 2 == 1:
                    nc.scalar.copy(dst_sl, ps[:, :bn * rowv])
                else:
                    nc.vector.tensor_copy(out=dst_sl, in_=ps[:, :bn * rowv])
                copy_tog += 1
                b += bn

            G4 = G[:].rearrange("p (t l v) -> p t l v", t=T, l=L, v=rowv)

            # ---- attention weights w = exp(lrelu(a_src + a_dst)) * mask
            E = epool.tile([P, T * L * H], f16, tag="E")
            E4 = E[:].rearrange("p (t l h) -> p t l h", t=T, l=L, h=H)
            nc.vector.tensor_tensor(
                out=E4,
                in0=G4[:, :, :, outf:outf + H],
                in1=OWND3[:, g0:g0 + T, :].unsqueeze(2)
                .to_broadcast([P, T, L, H]),
                op=mybir.AluOpType.add,
            )
            W = epool.tile([P, T * L * H], f16, tag="W")
            nc.vector.scalar_tensor_tensor(
                out=W[:], in0=E[:], scalar=NEG_SLOPE, in1=E[:],
                op0=mybir.AluOpType.mult, op1=mybir.AluOpType.max,
            )
            nc.scalar.activation(W[:], W[:], mybir.ActivationFunctionType.Exp)
            W4 = W[:].rearrange("p (t l h) -> p t l h", t=T, l=L, h=H)
            nc.vector.tensor_tensor(
                out=W4, in0=W4,
                in1=mask_sb[:, b0:b0 + nblk]
                .rearrange("p (t l) -> p t l", t=T, l=L)
                .unsqueeze(3).to_broadcast([P, T, L, H]),
                op=mybir.AluOpType.mult,
            )

            # ---- denominators + reciprocal
            den = epool.tile([P, T * H], f32, tag="den")
            den3 = den[:].rearrange("p (t h) -> p t h", t=T, h=H)
            nc.vector.tensor_reduce(
                out=den3, in_=W4.transpose([0, 1, 3, 2]),
                axis=mybir.AxisListType.X, op=mybir.AluOpType.add,
            )
            rec = epool.tile([P, T * H], f32, tag="rec")
            nc.vector.reciprocal(rec[:], den[:])
            rec3 = rec[:].rearrange("p (t h) -> p t h", t=T, h=H)

            # ---- weighted sum of h over slots
            gh4 = G4[:, :, :, :outf].rearrange(
                "p t l (c h) -> p t l c h", c=C, h=H)
            nc.vector.tensor_tensor(
                out=gh4, in0=gh4,
                in1=W4.unsqueeze(3).to_broadcast([P, T, L, C, H]),
                op=mybir.AluOpType.mult,
            )
            osb = epool.tile([P, T * outf], f32, tag="osb")
            osb3 = osb[:].rearrange("p (t f) -> p t f", t=T, f=outf)
            nc.vector.tensor_reduce(
                out=osb3,
                in_=G4[:, :, :, :outf].transpose([0, 1, 3, 2]),
                axis=mybir.AxisListType.X, op=mybir.AluOpType.add,
            )

            # ---- normalize + bias (+ relu), write out
            nc.vector.tensor_tensor(
                out=osb3.rearrange("p t (c h) -> p t c h", c=C, h=H),
                in0=osb3.rearrange("p t (c h) -> p t c h", c=C, h=H),
                in1=rec3.unsqueeze(2).to_broadcast([P, T, C, H]),
                op=mybir.AluOpType.mult,
            )
            nc.vector.tensor_tensor(
                out=osb3, in0=osb3,
                in1=bias_sb[:].unsqueeze(1).to_broadcast([P, T, outf]),
                op=mybir.AluOpType.add,
            )
            if relu:
                nc.scalar.activation(osb[:], osb[:],
                                     mybir.ActivationFunctionType.Relu)
            nc.sync.dma_start(
                out=out[g0 * P:(g0 + T) * P, :].rearrange(
                    "(t p) f -> p t f", t=T),
                in_=osb3,
            )

    nc.compile()
    return nc


# ------------------------------------------------------------------ execution


def _prep_wext(W, att_src, att_dst):
    """[fin, outf + 2H] fp16: [W (c-major cols) | W@a_src^T | W@a_dst^T]."""
    H, C = att_src.shape
    fin = W.shape[0]
    Wr = W.reshape(fin, H, C)
    a_s = np.einsum("fhc,hc->fh", Wr, att_src)
    a_d = np.einsum("fhc,hc->fh", Wr, att_dst)
    Wi = Wr.transpose(0, 2, 1).reshape(fin, H * C)  # (c, h) column order
    return np.concatenate([Wi, a_s, a_d], axis=1).astype(np.float16)


def _interleave_cols(v, H, C):
    return np.asarray(v, np.float32).reshape(H, C).T.reshape(H * C)


def _deinterleave(arr, H, C):
    n = arr.shape[0]
    return arr.reshape(n, C, H).transpose(0, 2, 1).reshape(n, H * C)


def run_layer(plan, nc, x_t, W, att_src, att_dst, b, n_cores):
    H, C = att_src.shape
    outf = H * C
    wext = _prep_wext(np.asarray(W, np.float32),
                      np.asarray(att_src, np.float32),
                      np.asarray(att_dst, np.float32))
    bias = np.broadcast_to(_interleave_cols(b, H, C), (P, outf)).copy()
    in_maps = [
        {"xsl": plan.expand(c, x_t), "xtown": plan.xtown(c, x_t),
         "wext": wext, "bias": bias, "mask": plan.masks[c]}
        for c in range(n_cores)
    ]
    trace = os.environ.get("GAT_TRACE", "") == "1"
    res = run_bass_kernel_spmd(nc, in_maps, list(range(n_cores)), trace=trace)
    if trace:
        LAST_EXEC_NS.append(res.exec_time_ns)
        LAST_RES.append(res)
    outs = [res.results[c]["out"] for c in range(n_cores)]
    return _deinterleave(plan.unpermute(outs, outf), H, C)


def gat_forward(x, edge_index, params, n_cores=N_CORES):
    x = np.asarray(x, np.float32)
    n = x.shape[0]
    ei = np.asarray(edge_index)

    plan = SlotPlan(ei[0], ei[1], n, n_cores, CFG["group"],
                    CFG["blk_budget"])
    W1, as1, ad1, b1, W2, as2, ad2, b2 = params

    x_t = x.T.astype(np.float16)
    nc1 = build_layer_program(plan, as1.shape[0], as1.shape[1],
                              relu=True, n_cores=n_cores)
    h = run_layer(plan, nc1, x_t, W1, as1, ad1, b1, n_cores)

    h_t = h.T.astype(np.float16)
    nc2 = build_layer_program(plan, as2.shape[0], as2.shape[1],
                              relu=False, n_cores=n_cores)
    out = run_layer(plan, nc2, h_t, W2, as2, ad2, b2, n_cores)
    return out


def kernel(x, edge_index, W1, att_src1, att_dst1, b1, W2, att_src2,
           att_dst2, b2):
    params = tuple(
        np.asarray(a, np.float32)
        for a in (W1, att_src1, att_dst1, b1, W2, att_src2, att_dst2, b2)
    )
    return gat_forward(x, edge_index, params).astype(np.float32)


# revision 7
# speedup vs baseline: 5.2994x; 1.6402x over previous
"""Trainium2 Bass kernel for a 2-layer GAT (PyG GATConv semantics).

Strategy (8 NeuronCores, SPMD, 2 launches = 1 per GAT layer):
  - Destinations sharded across cores (6272 per core incl. padding dsts),
    destinations degree-sorted so per-tile slot grids pad tightly.
  - NO device-side gather. The host slot-expands the layer input:
    xTsl[:, b*128 + j] = x^T column of the source of edge slot (b, j),
    where block b = (tile t, slot level l) and partition j = destination
    lane. Slot level 0 is the self-loop (PyG add_self_loops); levels
    1..deg are the in-edges; the rest are zero-padded (masked).
  - Each 128-column block becomes one PE matmul lhsT against
    wext = [W | W@a_src^T | W@a_dst^T], producing full per-edge rows
    [h | alpha_src | alpha_dst-of-src] directly in PSUM -- the same
    trick the previous version used for self-loop rows only, now for
    every edge. PSUM blocks are copied (batched per bank) to SBUF.
  - Attention: e = alpha_src(slot) + alpha_dst(dst) (dst alphas from a
    small per-tile matmul over own columns), w = exp(lrelu(e)) * mask;
    softmax is deferred: DVE reduces w and w*h over the slot axis, then
    one reciprocal multiply normalizes; + bias (+ relu for layer 1).
  - Between layers the host assembles h1, casts to fp16 and re-expands
    the SAME slot grid (graph is static), so layer 2 is identical with
    H=1, C=64.
"""

import sys

for _p in ("/opt/trn_rl_repo", "/root/.axon_site/_ro/trn_rl_repo"):
    if _p not in sys.path:
        sys.path.insert(0, _p)

import os
from contextlib import ExitStack

import numpy as np

import concourse.tile as tile
from concourse import bacc, mybir
from concourse.bass_utils import run_bass_kernel_spmd

# set GAT_TRACE=1 to profile each launch; exec times land in LAST_EXEC_NS
LAST_EXEC_NS = []
LAST_RES = []

CFG = {
    "group": 16,       # max tiles per group
    "blk_budget": 96,  # max T*L blocks per group (SBUF bound)
    "xsl_bufs": 2,
    "gpool_bufs": 2,
    "epool_bufs": 3,
    "psum_bufs": 6,
    "opsum_bufs": 2,
}

f32 = mybir.dt.float32
f16 = mybir.dt.float16

P = 128
NEG_SLOPE = 0.2
N_NODES = 50000
N_CORES = 8


# ---------------------------------------------------------------- host routing


class SlotPlan:
    """Destination-sharded slot grid; slot 0 = self-loop, then in-edges."""

    def __init__(self, src, dst, n_nodes, n_cores, group, blk_budget):
        self.n_nodes = n_nodes
        self.n_cores = n_cores
        self.dpc = int(np.ceil(n_nodes / n_cores / P)) * P
        self.nt = self.dpc // P
        nt = self.nt

        src = np.asarray(src, dtype=np.int64)
        dst = np.asarray(dst, dtype=np.int64)

        self.cores = []
        Ls = np.zeros(nt, np.int64)
        for c in range(n_cores):
            lo, hi = c * self.dpc, (c + 1) * self.dpc
            m = (dst >= lo) & (dst < hi)
            d_loc = (dst[m] - lo).astype(np.int64)
            s = src[m].astype(np.int64)
            order = np.argsort(d_loc, kind="stable")
            d_loc, s = d_loc[order], s[order]
            deg = np.bincount(d_loc, minlength=self.dpc)
            offs = np.zeros(self.dpc + 1, np.int64)
            np.cumsum(deg, out=offs[1:])
            perm = np.argsort(-deg, kind="stable").astype(np.int64)
            self.cores.append(dict(deg=deg, offs=offs, srcs=s, perm=perm))
            pt = deg[perm].reshape(nt, P)
            np.maximum(Ls, pt.max(axis=1) + 1, out=Ls)  # +1 self slot

        # SPMD-uniform groups: (g0, T) tiles sharing slot depth Lg
        self.groups = []
        t0 = 0
        while t0 < nt:
            T = 1
            while (
                T < group and t0 + T < nt
                and (T + 1) * int(Ls[t0:t0 + T + 1].max()) <= blk_budget
            ):
                T += 1
            self.groups.append((t0, T))
            t0 += T
        self.Lg = {g0: int(Ls[g0:g0 + T].max()) for g0, T in self.groups}
        self.n_blocks = sum(T * self.Lg[g0] for g0, T in self.groups)
        self.S = self.n_blocks * P  # total slot columns per core
        # block start offset per group
        self.gblk = {}
        b = 0
        for g0, T in self.groups:
            self.gblk[g0] = b
            b += T * self.Lg[g0]

        # per-core slot->node map (-1 = pad) and validity mask
        self.col_node = []
        self.masks = []
        for c in range(n_cores):
            st = self.cores[c]
            deg, offs, srcs, perm = (st["deg"], st["offs"], st["srcs"],
                                     st["perm"])
            cn = np.full((self.n_blocks, P), -1, np.int64)
            for g0, T in self.groups:
                L = self.Lg[g0]
                b0 = self.gblk[g0]
                for t in range(T):
                    dsts = perm[(g0 + t) * P:(g0 + t + 1) * P]
                    node = c * self.dpc + dsts
                    ok = node < n_nodes
                    # slot 0: self column (pad dst -> -1 column = zeros)
                    cn[b0 + t * L, ok] = node[ok]
                    for j in range(P):
                        d = dsts[j]
                        dd = deg[d]
                        if dd:
                            o = offs[d]
                            cn[b0 + t * L + 1:b0 + t * L + 1 + dd, j] = \
                                srcs[o:o + dd]
            self.col_node.append(cn)
            mask = (cn >= 0).astype(np.float16)
            # self slots of pad dsts: keep 1 so den=exp(0)=1 (row dropped)
            for g0, T in self.groups:
                L = self.Lg[g0]
                b0 = self.gblk[g0]
                for t in range(T):
                    mask[b0 + t * L, :] = 1.0
            self.masks.append(np.ascontiguousarray(mask.T))  # [P, n_blocks]

    def expand(self, core, x_t):
        """[128, S] f16: x^T columns in slot order; pad -> 0."""
        cn = self.col_node[core].reshape(-1)
        out = np.zeros((x_t.shape[0], cn.size), np.float16)
        ok = cn >= 0
        out[:, ok] = x_t[:, cn[ok]]
        return out

    def xtown(self, core, x_t):
        """[128, dpc] f16: own dst columns (A-order) for alpha_dst."""
        st = self.cores[core]
        node = core * self.dpc + st["perm"]
        valid = node < self.n_nodes
        out = np.zeros((x_t.shape[0], self.dpc), np.float16)
        out[:, valid] = x_t[:, node[valid]]
        return out

    def unpermute(self, core_outs, fout):
        full = np.zeros((self.n_nodes, fout), np.float32)
        for c, arr in enumerate(core_outs):
            node = c * self.dpc + self.cores[c]["perm"]
            m = node < self.n_nodes
            full[node[m]] = arr[m]
        return full


# ------------------------------------------------------------- device program


def build_layer_program(plan: SlotPlan, n_heads, ch, relu, n_cores):
    """One GAT layer over host-expanded slot columns. Returns compiled Bacc."""
    outf = n_heads * ch
    rowv = outf + 2 * n_heads  # [h | alpha_src | alpha_dst]
    nt = plan.nt
    H, C = n_heads, ch
    per_bank = 512 // rowv  # f32 psum cols per bank -> blocks per copy

    nc = bacc.Bacc(
        "TRN2",
        target_bir_lowering=False,
        debug=False,
        num_devices=n_cores,
    )
    xsl = nc.dram_tensor("xsl", [P, plan.S], f16, kind="ExternalInput").ap()
    xtown = nc.dram_tensor("xtown", [P, plan.dpc], f16,
                           kind="ExternalInput").ap()
    wext = nc.dram_tensor("wext", [P, rowv], f16, kind="ExternalInput").ap()
    maskin = nc.dram_tensor("mask", [P, plan.n_blocks], f16,
                            kind="ExternalInput").ap()
    bias = nc.dram_tensor("bias", [P, outf], f32, kind="ExternalInput").ap()
    ident_in = nc.dram_tensor("ident", [P, P], f16, kind="ExternalInput").ap()
    out = nc.dram_tensor("out", [plan.dpc, outf], f32,
                         kind="ExternalOutput").ap()
    tiles_per_bank = 512 // outf  # num-psum tiles per bank

    with tile.TileContext(nc) as tc, ExitStack() as ctx:
        const = ctx.enter_context(tc.tile_pool(name="const", bufs=1))
        xpool = ctx.enter_context(
            tc.tile_pool(name="xpool", bufs=CFG["xsl_bufs"]))
        gpool = ctx.enter_context(
            tc.tile_pool(name="gpool", bufs=CFG["gpool_bufs"]))
        epool = ctx.enter_context(
            tc.tile_pool(name="epool", bufs=CFG["epool_bufs"]))
        bpsum = ctx.enter_context(
            tc.tile_pool(name="bpsum", bufs=CFG["psum_bufs"], space="PSUM"))
        npsum = ctx.enter_context(
            tc.tile_pool(name="npsum", bufs=CFG["opsum_bufs"], space="PSUM"))

        ident = const.tile([P, P], f16)
        nc.sync.dma_start(out=ident[:], in_=ident_in[:])
        wext_sb = const.tile([P, rowv], f16)
        nc.sync.dma_start(out=wext_sb[:], in_=wext[:])
        bias_sb = const.tile([P, outf], f32)
        nc.sync.dma_start(out=bias_sb[:], in_=bias[:])
        mask_sb = const.tile([P, plan.n_blocks], f16)
        nc.sync.dma_start(out=mask_sb[:], in_=maskin[:])
        xtown_sb = const.tile([P, plan.dpc], f16)
        nc.sync.dma_start(out=xtown_sb[:], in_=xtown[:])

        # ---- OWND: alpha_dst of own dsts [P, nt*H] (one psum bank)
        OWND = const.tile([P, nt * H], f16)
        ps_d = npsum.tile([P, 512], f32, space="PSUM", tag="nps")
        for k in range(nt):
            nc.tensor.matmul(
                out=ps_d[:, k * H:(k + 1) * H],
                lhsT=xtown_sb[:, k * P:(k + 1) * P],
                rhs=wext_sb[:, outf + H:outf + 2 * H],
                start=True, stop=True,
            )
        nc.vector.tensor_copy(out=OWND[:], in_=ps_d[:, :nt * H])
        OWND3 = OWND[:].rearrange("p (t h) -> p t h", t=nt, h=H)

        # ---- per-group pipeline
        for g0, T in plan.groups:
            L = plan.Lg[g0]
            nblk = T * L
            b0 = plan.gblk[g0]

            xch = xpool.tile([P, nblk * P], f16, tag="xch")
            nc.sync.dma_start(
                out=xch[:], in_=xsl[:, b0 * P:(b0 + nblk) * P])

            G = gpool.tile([P, nblk * rowv], f16, tag="G")
            # build rows: one matmul per block, batched per psum bank
            b = 0
            copy_tog = 0
            while b < nblk:
                bn = min(per_bank, nblk - b)
                ps = bpsum.tile([P, 512], f32, space="PSUM", tag="bps")
                for k in range(bn):
                    nc.tensor.matmul(
                        out=ps[:, k * rowv:(k + 1) * rowv],
                        lhsT=xch[:, (b + k) * P:(b + k + 1) * P],
                        rhs=wext_sb[:],
                        start=True, stop=True,
                    )
                dst_sl = G[:, b * rowv:(b + bn) * rowv]
                if copy_tog % 2 == 1:
                    nc.scalar.copy(dst_sl, ps[:, :bn * rowv])
                else:
                    nc.vector.tensor_copy(out=dst_sl, in_=ps[:, :bn * rowv])
                copy_tog += 1
                b += bn

            G4 = G[:].rearrange("p (t l v) -> p t l v", t=T, l=L, v=rowv)

            # ---- attention weights w = exp(lrelu(a_src + a_dst)) * mask
            E = epool.tile([P, T * L * H], f16, tag="E")
            E4 = E[:].rearrange("p (t l h) -> p t l h", t=T, l=L, h=H)
            nc.vector.tensor_tensor(
                out=E4,
                in0=G4[:, :, :, outf:outf + H],
                in1=OWND3[:, g0:g0 + T, :].unsqueeze(2)
                .to_broadcast([P, T, L, H]),
                op=mybir.AluOpType.add,
            )
            W = epool.tile([P, T * L * H], f16, tag="W")
            nc.vector.scalar_tensor_tensor(
                out=W[:], in0=E[:], scalar=NEG_SLOPE, in1=E[:],
                op0=mybir.AluOpType.mult, op1=mybir.AluOpType.max,
            )
            nc.scalar.activation(W[:], W[:], mybir.ActivationFunctionType.Exp)
            W4 = W[:].rearrange("p (t l h) -> p t l h", t=T, l=L, h=H)
            nc.vector.tensor_tensor(
                out=W4, in0=W4,
                in1=mask_sb[:, b0:b0 + nblk]
                .rearrange("p (t l) -> p t l", t=T, l=L)
                .unsqueeze(3).to_broadcast([P, T, L, H]),
                op=mybir.AluOpType.mult,
            )

            # ---- denominators + reciprocal
            den = epool.tile([P, T * H], f32, tag="den")
            den3 = den[:].rearrange("p (t h) -> p t h", t=T, h=H)
            nc.vector.tensor_reduce(
                out=den3, in_=W4.transpose([0, 1, 3, 2]),
                axis=mybir.AxisListType.X, op=mybir.AluOpType.add,
            )
            rec = epool.tile([P, T * H], f32, tag="rec")
            nc.vector.reciprocal(rec[:], den[:])
            rec3 = rec[:].rearrange("p (t h) -> p t h", t=T, h=H)

            # ---- weighted sum of h over slots: scale on DVE, reduce on PE
            gh4 = G4[:, :, :, :outf].rearrange(
                "p t l (c h) -> p t l c h", c=C, h=H)
            nc.vector.tensor_tensor(
                out=gh4, in0=gh4,
                in1=W4.unsqueeze(3).to_broadcast([P, T, L, C, H]),
                op=mybir.AluOpType.mult,
            )
            osb = epool.tile([P, T * outf], f32, tag="osb")
            osb3 = osb[:].rearrange("p (t f) -> p t f", t=T, f=outf)
            G3 = G[:].rearrange("p (b v) -> p b v", b=T * L, v=rowv)
            t0 = 0
            while t0 < T:
                tn = min(tiles_per_bank, T - t0)
                ps = npsum.tile([P, 512], f32, space="PSUM", tag="nps")
                for tt in range(tn):
                    for j in range(L):
                        nc.tensor.matmul(
                            out=ps[:, tt * outf:(tt + 1) * outf],
                            lhsT=ident[:],
                            rhs=G3[:, (t0 + tt) * L + j, :outf],
                            start=(j == 0), stop=(j == L - 1),
                        )
                # normalize from PSUM: osb = num * (1/den)
                nc.vector.tensor_tensor(
                    out=osb3[:, t0:t0 + tn, :].rearrange(
                        "p t (c h) -> p t c h", c=C, h=H),
                    in0=ps[:, :tn * outf].rearrange(
                        "p (t c h) -> p t c h", t=tn, c=C, h=H),
                    in1=rec3[:, t0:t0 + tn, :].unsqueeze(2)
                    .to_broadcast([P, tn, C, H]),
                    op=mybir.AluOpType.mult,
                )
                t0 += tn

            # ---- bias (+ relu), write out
            nc.vector.tensor_tensor(
                out=osb3, in0=osb3,
                in1=bias_sb[:].unsqueeze(1).to_broadcast([P, T, outf]),
                op=mybir.AluOpType.add,
            )
            if relu:
                nc.scalar.activation(osb[:], osb[:],
                                     mybir.ActivationFunctionType.Relu)
            nc.sync.dma_start(
                out=out[g0 * P:(g0 + T) * P, :].rearrange(
                    "(t p) f -> p t f", t=T),
                in_=osb3,
            )

    nc.compile()
    return nc


# ------------------------------------------------------------------ execution


def _prep_wext(W, att_src, att_dst):
    """[fin, outf + 2H] fp16: [W (c-major cols) | W@a_src^T | W@a_dst^T]."""
    H, C = att_src.shape
    fin = W.shape[0]
    Wr = W.reshape(fin, H, C)
    a_s = np.einsum("fhc,hc->fh", Wr, att_src)
    a_d = np.einsum("fhc,hc->fh", Wr, att_dst)
    Wi = Wr.transpose(0, 2, 1).reshape(fin, H * C)  # (c, h) column order
    return np.concatenate([Wi, a_s, a_d], axis=1).astype(np.float16)


def _interleave_cols(v, H, C):
    return np.asarray(v, np.float32).reshape(H, C).T.reshape(H * C)


def _deinterleave(arr, H, C):
    n = arr.shape[0]
    return arr.reshape(n, C, H).transpose(0, 2, 1).reshape(n, H * C)


def run_layer(plan, nc, x_t, W, att_src, att_dst, b, n_cores):
    H, C = att_src.shape
    outf = H * C
    wext = _prep_wext(np.asarray(W, np.float32),
                      np.asarray(att_src, np.float32),
                      np.asarray(att_dst, np.float32))
    bias = np.broadcast_to(_interleave_cols(b, H, C), (P, outf)).copy()
    ident = np.eye(P, dtype=np.float16)
    in_maps = [
        {"xsl": plan.expand(c, x_t), "xtown": plan.xtown(c, x_t),
         "wext": wext, "bias": bias, "mask": plan.masks[c], "ident": ident}
        for c in range(n_cores)
    ]
    trace = os.environ.get("GAT_TRACE", "") == "1"
    res = run_bass_kernel_spmd(nc, in_maps, list(range(n_cores)), trace=trace)
    if trace:
        LAST_EXEC_NS.append(res.exec_time_ns)
        LAST_RES.append(res)
    outs = [res.results[c]["out"] for c in range(n_cores)]
    return _deinterleave(plan.unpermute(outs, outf), H, C)


def gat_forward(x, edge_index, params, n_cores=N_CORES):
    x = np.asarray(x, np.float32)
    n = x.shape[0]
    ei = np.asarray(edge_index)

    plan = SlotPlan(ei[0], ei[1], n, n_cores, CFG["group"],
                    CFG["blk_budget"])
    W1, as1, ad1, b1, W2, as2, ad2, b2 = params

    x_t = x.T.astype(np.float16)
    nc1 = build_layer_program(plan, as1.shape[0], as1.shape[1],
                              relu=True, n_cores=n_cores)
    h = run_layer(plan, nc1, x_t, W1, as1, ad1, b1, n_cores)

    h_t = h.T.astype(np.float16)
    nc2 = build_layer_program(plan, as2.shape[0], as2.shape[1],
                              relu=False, n_cores=n_cores)
    out = run_layer(plan, nc2, h_t, W2, as2, ad2, b2, n_cores)
    return out


def kernel(x, edge_index, W1, att_src1, att_dst1, b1, W2, att_src2,
           att_dst2, b2):
    params = tuple(
        np.asarray(a, np.float32)
        for a in (W1, att_src1, att_dst1, b1, W2, att_src2, att_dst2, b2)
    )
    return gat_forward(x, edge_index, params).astype(np.float32)


# revision 11
# speedup vs baseline: 5.9153x; 1.1162x over previous
"""Trainium2 Bass kernel for a 2-layer GAT (PyG GATConv semantics).

Strategy (8 NeuronCores, SPMD, 2 launches = 1 per GAT layer):
  - Destinations sharded across cores (6272 per core incl. padding dsts),
    destinations degree-sorted so per-tile slot grids pad tightly.
  - NO device-side gather. The host slot-expands the layer input:
    xTsl[:, b*128 + j] = x^T column of the source of edge slot (b, j),
    where block b = (tile t, slot level l) and partition j = destination
    lane. Slot level 0 is the self-loop (PyG add_self_loops); levels
    1..deg are the in-edges; the rest are zero-padded (masked).
  - Each 128-column block becomes one PE matmul lhsT against
    wext = [W | W@a_src^T | W@a_dst^T], producing full per-edge rows
    [h | alpha_src | alpha_dst-of-src] directly in PSUM -- the same
    trick the previous version used for self-loop rows only, now for
    every edge. PSUM blocks are copied (batched per bank) to SBUF.
  - Attention: e = alpha_src(slot) + alpha_dst(dst) (dst alphas from a
    small per-tile matmul over own columns), w = exp(lrelu(e)) * mask;
    softmax is deferred: DVE reduces w and w*h over the slot axis, then
    one reciprocal multiply normalizes; + bias (+ relu for layer 1).
  - Between layers the host assembles h1, casts to fp16 and re-expands
    the SAME slot grid (graph is static), so layer 2 is identical with
    H=1, C=64.
"""

import sys

for _p in ("/opt/trn_rl_repo", "/root/.axon_site/_ro/trn_rl_repo"):
    if _p not in sys.path:
        sys.path.insert(0, _p)

import os
from contextlib import ExitStack

import numpy as np

import concourse.tile as tile
from concourse import bacc, mybir
from concourse.bass_utils import run_bass_kernel_spmd

# set GAT_TRACE=1 to profile each launch; exec times land in LAST_EXEC_NS
LAST_EXEC_NS = []
LAST_RES = []

CFG = {
    "group": 16,       # max tiles per group
    "blk_budget": 96,  # max T*L blocks per group (SBUF bound)
    "xsl_bufs": 2,
    "gpool_bufs": 2,
    "epool_bufs": 3,
    "psum_bufs": 3,  # bpsum tiles are 2 banks each
    "opsum_bufs": 2,
}

f32 = mybir.dt.float32
f16 = mybir.dt.float16

P = 128
NEG_SLOPE = 0.2
N_NODES = 50000
N_CORES = 8


# ---------------------------------------------------------------- host routing


class SlotPlan:
    """Destination-sharded slot grid; slot 0 = self-loop, then in-edges."""

    def __init__(self, src, dst, n_nodes, n_cores, group, blk_budget):
        self.n_nodes = n_nodes
        self.n_cores = n_cores
        self.dpc = int(np.ceil(n_nodes / n_cores / P)) * P
        self.nt = self.dpc // P
        nt = self.nt

        src = np.asarray(src, dtype=np.int64)
        dst = np.asarray(dst, dtype=np.int64)

        self.cores = []
        Ls = np.zeros(nt, np.int64)
        for c in range(n_cores):
            lo, hi = c * self.dpc, (c + 1) * self.dpc
            m = (dst >= lo) & (dst < hi)
            d_loc = (dst[m] - lo).astype(np.int64)
            s = src[m].astype(np.int64)
            order = np.argsort(d_loc, kind="stable")
            d_loc, s = d_loc[order], s[order]
            deg = np.bincount(d_loc, minlength=self.dpc)
            offs = np.zeros(self.dpc + 1, np.int64)
            np.cumsum(deg, out=offs[1:])
            perm = np.argsort(-deg, kind="stable").astype(np.int64)
            self.cores.append(dict(deg=deg, offs=offs, srcs=s, perm=perm))
            pt = deg[perm].reshape(nt, P)
            np.maximum(Ls, pt.max(axis=1) + 1, out=Ls)  # +1 self slot

        # SPMD-uniform groups: (g0, T) tiles sharing slot depth Lg
        self.groups = []
        t0 = 0
        while t0 < nt:
            T = 1
            while (
                T < group and t0 + T < nt
                and (T + 1) * int(Ls[t0:t0 + T + 1].max()) <= blk_budget
            ):
                T += 1
            self.groups.append((t0, T))
            t0 += T
        self.Lg = {g0: int(Ls[g0:g0 + T].max()) for g0, T in self.groups}
        self.n_blocks = sum(T * self.Lg[g0] for g0, T in self.groups)
        self.S = self.n_blocks * P  # total slot columns per core
        # block start offset per group
        self.gblk = {}
        b = 0
        for g0, T in self.groups:
            self.gblk[g0] = b
            b += T * self.Lg[g0]

        # per-core slot->node map (-1 = pad) and validity mask
        self.col_node = []
        self.masks = []
        for c in range(n_cores):
            st = self.cores[c]
            deg, offs, srcs, perm = (st["deg"], st["offs"], st["srcs"],
                                     st["perm"])
            cn = np.full((self.n_blocks, P), -1, np.int64)
            for g0, T in self.groups:
                L = self.Lg[g0]
                b0 = self.gblk[g0]
                for t in range(T):
                    dsts = perm[(g0 + t) * P:(g0 + t + 1) * P]
                    node = c * self.dpc + dsts
                    ok = node < n_nodes
                    # slot 0: self column (pad dst -> -1 column = zeros)
                    cn[b0 + t * L, ok] = node[ok]
                    for j in range(P):
                        d = dsts[j]
                        dd = deg[d]
                        if dd:
                            o = offs[d]
                            cn[b0 + t * L + 1:b0 + t * L + 1 + dd, j] = \
                                srcs[o:o + dd]
            self.col_node.append(cn)
            mask = (cn >= 0).astype(np.float16)
            # self slots of pad dsts: keep 1 so den=exp(0)=1 (row dropped)
            for g0, T in self.groups:
                L = self.Lg[g0]
                b0 = self.gblk[g0]
                for t in range(T):
                    mask[b0 + t * L, :] = 1.0
            self.masks.append(np.ascontiguousarray(mask.T))  # [P, n_blocks]

    def expand(self, core, x_t):
        """[128, S] f16: x^T columns in slot order; pad -> 0."""
        cn = self.col_node[core].reshape(-1)
        out = np.zeros((x_t.shape[0], cn.size), np.float16)
        ok = cn >= 0
        out[:, ok] = x_t[:, cn[ok]]
        return out

    def xtown(self, core, x_t):
        """[128, dpc] f16: own dst columns (A-order) for alpha_dst."""
        st = self.cores[core]
        node = core * self.dpc + st["perm"]
        valid = node < self.n_nodes
        out = np.zeros((x_t.shape[0], self.dpc), np.float16)
        out[:, valid] = x_t[:, node[valid]]
        return out

    def unpermute(self, core_outs, fout):
        full = np.zeros((self.n_nodes, fout), np.float32)
        for c, arr in enumerate(core_outs):
            node = c * self.dpc + self.cores[c]["perm"]
            m = node < self.n_nodes
            full[node[m]] = arr[m]
        return full


# ------------------------------------------------------------- device program


def build_layer_program(plan: SlotPlan, n_heads, ch, relu, n_cores):
    """One GAT layer over host-expanded slot columns. Returns compiled Bacc."""
    outf = n_heads * ch
    rowv = outf + n_heads  # built rows: [h | alpha_src]
    wcols = outf + 2 * n_heads  # wext input: [W | a_src | a_dst]
    nt = plan.nt
    H, C = n_heads, ch
    per_bank = 1024 // rowv  # f32 psum cols per 2 banks -> blocks per copy

    nc = bacc.Bacc(
        "TRN2",
        target_bir_lowering=False,
        debug=False,
        num_devices=n_cores,
    )
    xsl = nc.dram_tensor("xsl", [P, plan.S], f16, kind="ExternalInput").ap()
    xtown = nc.dram_tensor("xtown", [P, plan.dpc], f16,
                           kind="ExternalInput").ap()
    wext = nc.dram_tensor("wext", [P, wcols], f16, kind="ExternalInput").ap()
    maskin = nc.dram_tensor("mask", [P, plan.n_blocks], f16,
                            kind="ExternalInput").ap()
    bias = nc.dram_tensor("bias", [P, outf], f32, kind="ExternalInput").ap()
    ident_in = nc.dram_tensor("ident", [P, P], f16, kind="ExternalInput").ap()
    out = nc.dram_tensor("out", [plan.dpc, outf], f32,
                         kind="ExternalOutput").ap()
    tiles_per_bank = 512 // outf  # num-psum tiles per bank

    with tile.TileContext(nc) as tc, ExitStack() as ctx:
        const = ctx.enter_context(tc.tile_pool(name="const", bufs=1))
        xpool = ctx.enter_context(
            tc.tile_pool(name="xpool", bufs=CFG["xsl_bufs"]))
        gpool = ctx.enter_context(
            tc.tile_pool(name="gpool", bufs=CFG["gpool_bufs"]))
        epool = ctx.enter_context(
            tc.tile_pool(name="epool", bufs=CFG["epool_bufs"]))
        bpsum = ctx.enter_context(
            tc.tile_pool(name="bpsum", bufs=CFG["psum_bufs"], space="PSUM"))
        npsum = ctx.enter_context(
            tc.tile_pool(name="npsum", bufs=CFG["opsum_bufs"], space="PSUM"))

        ident = const.tile([P, P], f16)
        nc.sync.dma_start(out=ident[:], in_=ident_in[:])
        wext_sb = const.tile([P, wcols], f16)
        nc.sync.dma_start(out=wext_sb[:], in_=wext[:])
        bias_sb = const.tile([P, outf], f32)
        nc.sync.dma_start(out=bias_sb[:], in_=bias[:])
        mask_sb = const.tile([P, plan.n_blocks], f16)
        nc.sync.dma_start(out=mask_sb[:], in_=maskin[:])
        xtown_sb = const.tile([P, plan.dpc], f16)
        nc.sync.dma_start(out=xtown_sb[:], in_=xtown[:])

        # ---- OWND: alpha_dst of own dsts [P, nt*H] (one psum bank)
        OWND = const.tile([P, nt * H], f16)
        ps_d = npsum.tile([P, 512], f32, space="PSUM", tag="nps")
        for k in range(nt):
            nc.tensor.matmul(
                out=ps_d[:, k * H:(k + 1) * H],
                lhsT=xtown_sb[:, k * P:(k + 1) * P],
                rhs=wext_sb[:, outf + H:outf + 2 * H],
                start=True, stop=True,
            )
        nc.vector.tensor_copy(out=OWND[:], in_=ps_d[:, :nt * H])
        OWND3 = OWND[:].rearrange("p (t h) -> p t h", t=nt, h=H)

        # ---- per-group pipeline
        for g0, T in plan.groups:
            L = plan.Lg[g0]
            nblk = T * L
            b0 = plan.gblk[g0]

            xch = xpool.tile([P, nblk * P], f16, tag="xch")
            nc.sync.dma_start(
                out=xch[:], in_=xsl[:, b0 * P:(b0 + nblk) * P])

            G = gpool.tile([P, nblk * rowv], f16, tag="G")
            # build rows: one matmul per block, batched per psum bank
            b = 0
            while b < nblk:
                bn = min(per_bank, nblk - b)
                ps = bpsum.tile([P, 1024], f32, space="PSUM", tag="bps")
                for k in range(bn):
                    nc.tensor.matmul(
                        out=ps[:, k * rowv:(k + 1) * rowv],
                        lhsT=xch[:, (b + k) * P:(b + k + 1) * P],
                        rhs=wext_sb[:, :rowv],
                        start=True, stop=True,
                    )
                nc.scalar.copy(G[:, b * rowv:(b + bn) * rowv],
                               ps[:, :bn * rowv])
                b += bn

            G4 = G[:].rearrange("p (t l v) -> p t l v", t=T, l=L, v=rowv)

            # ---- attention weights w = exp(lrelu(a_src + a_dst)) * mask
            E = epool.tile([P, T * L * H], f16, tag="E")
            E4 = E[:].rearrange("p (t l h) -> p t l h", t=T, l=L, h=H)
            nc.vector.tensor_tensor(
                out=E4,
                in0=G4[:, :, :, outf:outf + H],
                in1=OWND3[:, g0:g0 + T, :].unsqueeze(2)
                .to_broadcast([P, T, L, H]),
                op=mybir.AluOpType.add,
            )
            W = epool.tile([P, T * L * H], f16, tag="W")
            nc.vector.scalar_tensor_tensor(
                out=W[:], in0=E[:], scalar=NEG_SLOPE, in1=E[:],
                op0=mybir.AluOpType.mult, op1=mybir.AluOpType.max,
            )
            nc.scalar.activation(W[:], W[:], mybir.ActivationFunctionType.Exp)
            W4 = W[:].rearrange("p (t l h) -> p t l h", t=T, l=L, h=H)
            nc.vector.tensor_tensor(
                out=W4, in0=W4,
                in1=mask_sb[:, b0:b0 + nblk]
                .rearrange("p (t l) -> p t l", t=T, l=L)
                .unsqueeze(3).to_broadcast([P, T, L, H]),
                op=mybir.AluOpType.mult,
            )

            # ---- denominators + reciprocal
            den = epool.tile([P, T * H], f32, tag="den")
            den3 = den[:].rearrange("p (t h) -> p t h", t=T, h=H)
            nc.vector.tensor_reduce(
                out=den3, in_=W4.transpose([0, 1, 3, 2]),
                axis=mybir.AxisListType.X, op=mybir.AluOpType.add,
            )
            rec = epool.tile([P, T * H], f32, tag="rec")
            nc.vector.reciprocal(rec[:], den[:])
            rec3 = rec[:].rearrange("p (t h) -> p t h", t=T, h=H)

            # ---- weighted sum of h over slots: scale on DVE, reduce on PE
            gh4 = G4[:, :, :, :outf].rearrange(
                "p t l (c h) -> p t l c h", c=C, h=H)
            nc.vector.tensor_tensor(
                out=gh4, in0=gh4,
                in1=W4.unsqueeze(3).to_broadcast([P, T, L, C, H]),
                op=mybir.AluOpType.mult,
            )
            osb = epool.tile([P, T * outf], f32, tag="osb")
            osb3 = osb[:].rearrange("p (t f) -> p t f", t=T, f=outf)
            G3 = G[:].rearrange("p (b v) -> p b v", b=T * L, v=rowv)
            t0 = 0
            while t0 < T:
                tn = min(tiles_per_bank, T - t0)
                ps = npsum.tile([P, 512], f32, space="PSUM", tag="nps")
                for tt in range(tn):
                    for j in range(L):
                        nc.tensor.matmul(
                            out=ps[:, tt * outf:(tt + 1) * outf],
                            lhsT=ident[:],
                            rhs=G3[:, (t0 + tt) * L + j, :outf],
                            start=(j == 0), stop=(j == L - 1),
                        )
                # normalize from PSUM: osb = num * (1/den)
                nc.vector.tensor_tensor(
                    out=osb3[:, t0:t0 + tn, :].rearrange(
                        "p t (c h) -> p t c h", c=C, h=H),
                    in0=ps[:, :tn * outf].rearrange(
                        "p (t c h) -> p t c h", t=tn, c=C, h=H),
                    in1=rec3[:, t0:t0 + tn, :].unsqueeze(2)
                    .to_broadcast([P, tn, C, H]),
                    op=mybir.AluOpType.mult,
                )
                t0 += tn

            # ---- bias (+ relu), write out
            nc.vector.tensor_tensor(
                out=osb3, in0=osb3,
                in1=bias_sb[:].unsqueeze(1).to_broadcast([P, T, outf]),
                op=mybir.AluOpType.add,
            )
            if relu:
                nc.scalar.activation(osb[:], osb[:],
                                     mybir.ActivationFunctionType.Relu)
            nc.sync.dma_start(
                out=out[g0 * P:(g0 + T) * P, :].rearrange(
                    "(t p) f -> p t f", t=T),
                in_=osb3,
            )

    nc.compile()
    return nc


# ------------------------------------------------------------------ execution


def _prep_wext(W, att_src, att_dst):
    """[fin, outf + 2H] fp16: [W (c-major cols) | W@a_src^T | W@a_dst^T]."""
    H, C = att_src.shape
    fin = W.shape[0]
    Wr = W.reshape(fin, H, C)
    a_s = np.einsum("fhc,hc->fh", Wr, att_src)
    a_d = np.einsum("fhc,hc->fh", Wr, att_dst)
    Wi = Wr.transpose(0, 2, 1).reshape(fin, H * C)  # (c, h) column order
    return np.concatenate([Wi, a_s, a_d], axis=1).astype(np.float16)


def _interleave_cols(v, H, C):
    return np.asarray(v, np.float32).reshape(H, C).T.reshape(H * C)


def _deinterleave(arr, H, C):
    n = arr.shape[0]
    return arr.reshape(n, C, H).transpose(0, 2, 1).reshape(n, H * C)


def run_layer(plan, nc, x_t, W, att_src, att_dst, b, n_cores):
    H, C = att_src.shape
    outf = H * C
    wext = _prep_wext(np.asarray(W, np.float32),
                      np.asarray(att_src, np.float32),
                      np.asarray(att_dst, np.float32))
    bias = np.broadcast_to(_interleave_cols(b, H, C), (P, outf)).copy()
    ident = np.eye(P, dtype=np.float16)
    in_maps = [
        {"xsl": plan.expand(c, x_t), "xtown": plan.xtown(c, x_t),
         "wext": wext, "bias": bias, "mask": plan.masks[c], "ident": ident}
        for c in range(n_cores)
    ]
    trace = os.environ.get("GAT_TRACE", "") == "1"
    res = run_bass_kernel_spmd(nc, in_maps, list(range(n_cores)), trace=trace)
    if trace:
        LAST_EXEC_NS.append(res.exec_time_ns)
        LAST_RES.append(res)
    outs = [res.results[c]["out"] for c in range(n_cores)]
    return _deinterleave(plan.unpermute(outs, outf), H, C)


def gat_forward(x, edge_index, params, n_cores=N_CORES):
    x = np.asarray(x, np.float32)
    n = x.shape[0]
    ei = np.asarray(edge_index)

    plan = SlotPlan(ei[0], ei[1], n, n_cores, CFG["group"],
                    CFG["blk_budget"])
    W1, as1, ad1, b1, W2, as2, ad2, b2 = params

    x_t = x.T.astype(np.float16)
    nc1 = build_layer_program(plan, as1.shape[0], as1.shape[1],
                              relu=True, n_cores=n_cores)
    h = run_layer(plan, nc1, x_t, W1, as1, ad1, b1, n_cores)

    h_t = h.T.astype(np.float16)
    nc2 = build_layer_program(plan, as2.shape[0], as2.shape[1],
                              relu=False, n_cores=n_cores)
    out = run_layer(plan, nc2, h_t, W2, as2, ad2, b2, n_cores)
    return out


def kernel(x, edge_index, W1, att_src1, att_dst1, b1, W2, att_src2,
           att_dst2, b2):
    params = tuple(
        np.asarray(a, np.float32)
        for a in (W1, att_src1, att_dst1, b1, W2, att_src2, att_dst2, b2)
    )
    return gat_forward(x, edge_index, params).astype(np.float32)
